# revision 35
# baseline (speedup 1.0000x reference)
"""CompressedLinear on 8 Trainium2 NeuronCores.

out[b,s,o] = sum_i x[b,s,i] * (w_int8[o,i] * scale[o]) + bias[o]
  x: [4, 2048, 4096] f32, w_int8: [16384, 4096] int32 (codes in [-64,63]),
  scale/bias: [16384] f32 -> out: [4, 2048, 16384] f32

Strategy (tensor-parallel over out_features + mixed-precision by |scale|):
  - Each of the 8 cores owns a 2048-row slice of W/scale/bias and computes
    out[:, :, cols]; x is replicated. Rows are globally PERMUTED by |scale|
    (host-side, inverted on gather): the error metric is an L2 norm over the
    output, and a row's contribution is weighted by scale[o]^2, so the
    smallest-|scale| rows tolerate much cruder arithmetic.
  - Per core, PSUM banks 0-1 (the 1024 globally-smallest-|scale| rows of
    this core's share) run entirely in fp8e4 (e4m3) with
    perf_mode=DoubleRow: 2 fp8 weights per PE cell, 2 MACs/cell/cycle ->
    2x the bf16 matmul rate (measured: same ~216ns issue gap per MM for 2x
    the contraction). Bank 2 (mid rows) is MIXED: k-tiles 0..MIXK8-1 in
    fp8 DoubleRow, the rest bf16, one accumulation group. Bank 3 (largest
    rows) is pure bf16 with EXACT integer codes (|codes|<=64 fits bf16's
    8-bit significand). Per-tile matmul count: 32+23+32 = 87 vs 128
    all-bf16.
  - Codes going to fp8 are pre-scaled by RSC=1.0125 (folded back via the
    post-matmul scale): aligns the int grid to the e4m3 grid ~11% better.
  - scale is applied POST-matmul (PSUM accumulates x*codes), so bf16 work
    only carries x's bf16 rounding (~1.7e-3) and the fp8 work ~3.5e-2;
    scale^2-weighting puts the total at 1.906e-2 vs the 2e-2 gate
    (host-side numpy sim of the exact quantization reproduces the HW
    relative error to 6 significant digits; max-element-relative is
    1.893e-2, also clear of the gate).
  - A bf16<->DoubleRow perf-mode switch costs ~350ns on the PE, so the
    bf16/fp8 sections alternate order by tile parity (bf16-first on even
    tiles, fp8-first on odd) -> mode changes only once per tile.
  - Epilogue per token tile: ob = ps*scale_bcast + bias_bcast (2 DVE ops),
    then DMA store; fully overlapped with PE. The LAST tile is emitted
    bank-by-bank (bf16 banks first) with per-bank epilogue+store so almost
    nothing serializes after the final matmul.
  - w is DMA'd in per-k chunks so the t=0 matmuls ride the w stream.

All data layout transforms (transpose, dtype casts, row permutation,
scale/bias broadcast) are host-side numpy; gather inverts the permutation.
HW exec: ~1.259ms vs 1.803ms for the all-bf16 baseline (PE roofline for
all-bf16 is ~1.775ms; this kernel's matmul-count floor is ~1.206ms).
"""

import os

import numpy as np
import ml_dtypes

BF16 = ml_dtypes.bfloat16
E4M3 = ml_dtypes.float8_e4m3

OUT, IN = 16384, 4096
B, S = 4, 2048
TOK = B * S            # 8192 tokens
NCORES = 8
OSH = OUT // NCORES    # 2048 out-features per core
KT = IN // 128         # 32 k-tiles
TT = TOK // 128        # 64 token tiles
NB = OSH // 512        # 4 psum banks per token tile
NF8 = 2                # banks 0..NF8-1 are pure fp8 DoubleRow
KP = KT // 2           # 16 k-pairs for DoubleRow
MIXP = 9               # mixed bank (bank 2): k-pairs 0..MIXP-1 fp8, rest bf16
MIXK8 = 2 * MIXP       # k-tiles of the mixed bank in fp8
MIXB = NF8             # index of the mixed bank
XBUFS = 3              # x tile triple buffering (early-pipeline slack)
# Global pre-scaler for every fp8/scaled weight: int codes quantize to the
# e4m3 grid ~11% more accurately at this alignment; 1/R is folded into the
# post-matmul scale (exact), and the mixed bank's bf16 codes are scaled by
# R too (bf16 rounding of codes*R is ~2e-3, negligible at those rows).
RSC = 1.0125

_last_results = None   # BassKernelResults of the most recent run (for test.py)


def _build_program():
    from contextlib import ExitStack

    import concourse.bass as bass
    import concourse.tile as tile
    from concourse import mybir

    f32 = mybir.dt.float32
    bf16 = mybir.dt.bfloat16
    f8e4 = mybir.dt.float8e4
    DR = mybir.MatmulPerfMode.DoubleRow

    N8 = NF8 * 512           # pure-fp8 out-cols per core

    nc = bass.Bass()
    x16_d = nc.declare_dram_parameter("x16", [TT, 128, KT, 128], bf16, isOutput=False)
    x8_d = nc.declare_dram_parameter("x8", [TT, 128, KT, 128], f8e4, isOutput=False)
    # bank 3 (pure bf16) and bank 2's bf16 k-range
    w16_d = nc.declare_dram_parameter("w16", [128, KT, 512], bf16, isOutput=False)
    w16m_d = nc.declare_dram_parameter(
        "w16m", [128, KT - MIXK8, 512], bf16, isOutput=False
    )
    w8_d = nc.declare_dram_parameter("w8", [128, KT, N8], f8e4, isOutput=False)
    w8m_d = nc.declare_dram_parameter("w8m", [128, MIXK8, 512], f8e4, isOutput=False)
    sc_d = nc.declare_dram_parameter("sc", [128, NB, 512], f32, isOutput=False)
    bi_d = nc.declare_dram_parameter("bi", [128, NB, 512], f32, isOutput=False)
    out_d = nc.declare_dram_parameter("out", [TT, 128, NB, 512], f32, isOutput=True)

    from concourse.tile import add_dep_helper

    with tile.TileContext(nc) as tc, ExitStack() as ctx:
        wpool = ctx.enter_context(tc.tile_pool(name="w", bufs=1))
        cpool = ctx.enter_context(tc.tile_pool(name="consts", bufs=1))
        xpool = ctx.enter_context(tc.tile_pool(name="x", bufs=XBUFS))
        opool = ctx.enter_context(tc.tile_pool(name="o", bufs=2))
        pspool = ctx.enter_context(tc.tile_pool(name="ps", bufs=2, space="PSUM"))

        # w chunked so the first matmuls ride the w stream. Stream order
        # matches t=0 consumption: bank3 k-chunks (with bank2's bf16 chunks
        # interleaved at matching k), then bank2's fp8 pairs, then the pure
        # fp8 banks' pairs, then the epilogue consts.
        w16_sb = wpool.tile([128, KT, 512], bf16, tag="w16")
        w16m_sb = wpool.tile([128, KT - MIXK8, 512], bf16, tag="w16m")
        w8_sb = wpool.tile([128, KT, N8], f8e4, tag="w8")
        w8m_sb = wpool.tile([128, MIXK8, 512], f8e4, tag="w8m")
        w_dmas = []
        for k in range(KT):
            w_dmas.append(nc.sync.dma_start(w16_sb[:, k, :], w16_d[:, k, :]))
            if k >= MIXK8:
                w_dmas.append(
                    nc.sync.dma_start(
                        w16m_sb[:, k - MIXK8, :], w16m_d[:, k - MIXK8, :]
                    )
                )
        for kp in range(MIXP):
            w_dmas.append(
                nc.sync.dma_start(
                    w8m_sb[:, 2 * kp : 2 * kp + 2, :],
                    w8m_d[:, 2 * kp : 2 * kp + 2, :],
                )
            )
        for kp in range(KP):
            w_dmas.append(
                nc.sync.dma_start(
                    w8_sb[:, 2 * kp : 2 * kp + 2, :], w8_d[:, 2 * kp : 2 * kp + 2, :]
                )
            )
        sc_sb = cpool.tile([128, NB, 512], f32, tag="sc")
        sc_dma = nc.sync.dma_start(sc_sb[:], sc_d[:])
        bi_sb = cpool.tile([128, NB, 512], f32, tag="bi")
        bi_dma = nc.sync.dma_start(bi_sb[:], bi_d[:])
        hwdge_all = w_dmas + [sc_dma, bi_dma]

        # Per-iteration disjoint scratch columns -> the carrier ops carry no
        # WAW deps of their own.
        scratch = cpool.tile([1, TT], f32, tag="scratch")
        dummy = cpool.tile([1, 3 * TT], f32, tag="dummy")
        dummy2 = cpool.tile([1, 4 * TT], f32, tag="dummy2")
        dummy3 = cpool.tile([1, 8], f32, tag="dummy3")  # final-tile POOL carriers
        dveA = cpool.tile([1, TT], f32, tag="dveA")
        dveB = cpool.tile([1, TT], f32, tag="dveB")
        dveC = cpool.tile([1, TT], f32, tag="dveC")
        # Preamble DVE carriers: observe the sc/bi const loads on DVE so no
        # steady-state DVE op pairs a DMAHW wait with another wait.
        pre = cpool.tile([1, 8], f32, tag="pre")
        nc.vector.tensor_copy(pre[:, 0:1], sc_sb[:1, 0, :1])
        nc.vector.tensor_copy(pre[:, 2:3], bi_sb[:1, 0, :1])

        psum_readers = []  # the scale-mult (last psum reader) per iteration
        last_mms = []  # final matmul per iteration
        out_dmas = []
        out_copies = []
        x_dmas = []
        adds = []
        swdge_all = []  # every SWDGE DMA in emission order (tail coverage)

        # Hardware sync-wait slots are tiny (1 per PE LW/MM and per SWDGE
        # DMA, 2 per HWDGE DMA), and Tile's wait assignment is per-proc
        # minimal but not transitive. So every cross-engine dependency is
        # absorbed by a dedicated cheap "carrier" op on the consuming engine,
        # with explicit ordering edges so the scheduler keeps each carrier
        # ahead of its dependents and every instruction introduces at most
        # one new wait.
        def order(after, before):
            add_dep_helper(after.ins, before.ins, sync=False, reason="carrier order")

        def emit_mms(ps, x16, x8, banks, bf16_first=True):
            """Emit the matmuls for the given psum banks; returns
            (first_mm, last_mm, last_bank).

            bank 3: pure bf16; bank 2: bf16 k>=MIXK8 + fp8 pairs
            0..MIXP-1 (one accumulation group mixing modes); banks 0..1:
            pure fp8 DoubleRow. `bf16_first` selects the section order:
            alternating it per token tile keeps consecutive tiles in the
            same perf mode at the boundary (each bf16<->DoubleRow switch
            costs ~350ns on the PE).
            """
            state = {"first": None, "last": None, "bank": None}

            def mm(bank, *args, **kw):
                m = nc.tensor.matmul(*args, **kw)
                if state["first"] is None:
                    state["first"] = m
                state["last"] = m
                state["bank"] = bank
                return m

            def bf16_section(mix_started):
                for k in range(KT):
                    if 3 in banks:
                        mm(
                            3,
                            ps[:, 3, :],
                            x16[:, k, :],
                            w16_sb[:, k, :],
                            start=(k == 0),
                            stop=(k == KT - 1),
                        )
                    if MIXB in banks and k >= MIXK8:
                        mm(
                            MIXB,
                            ps[:, MIXB, :],
                            x16[:, k, :],
                            w16m_sb[:, k - MIXK8, :],
                            start=(k == MIXK8 and not mix_started),
                            stop=(k == KT - 1 and mix_started),
                        )

            def dr_section(mix_started):
                if MIXB in banks:
                    for kp in range(MIXP):
                        mm(
                            MIXB,
                            ps[:, MIXB, :],
                            x8[:, 2 * kp : 2 * kp + 2, :],
                            w8m_sb[:, 2 * kp : 2 * kp + 2, :],
                            start=(kp == 0 and not mix_started),
                            stop=(kp == MIXP - 1 and mix_started),
                            perf_mode=DR,
                        )
                for kp in range(KP):
                    for j in range(NF8):
                        if j in banks:
                            mm(
                                j,
                                ps[:, j, :],
                                x8[:, 2 * kp : 2 * kp + 2, :],
                                w8_sb[
                                    :, 2 * kp : 2 * kp + 2, j * 512 : (j + 1) * 512
                                ],
                                start=(kp == 0),
                                stop=(kp == KP - 1),
                                perf_mode=DR,
                            )

            if bf16_first:
                bf16_section(mix_started=False)
                dr_section(mix_started=True)
            else:
                dr_section(mix_started=False)
                bf16_section(mix_started=True)
            return state["first"], state["last"], state["bank"]

        for t in range(TT):
            x16 = xpool.tile([128, KT, 128], bf16, tag="x16")
            x8 = xpool.tile([128, KT, 128], f8e4, tag="x8")
            # POOL carrier chain, one wait each: gen-2 x-load DMA(s) (their
            # lane sems would otherwise ride the new DMA as WAW waits) and
            # gen-2 matmul (x slot reader), before the x-slot rewrite; the
            # same-engine program order shields the gpsimd dma_start.
            ms1 = nc.gpsimd.memset(dummy[:, 3 * t : 3 * t + 1], 0)
            ms3 = nc.gpsimd.memset(dummy[:, 3 * t + 2 : 3 * t + 3], 0)
            order(ms3, ms1)
            if t >= XBUFS:
                prev = x_dmas[t - XBUFS]
                add_dep_helper(
                    ms1.ins, prev[-1].ins, reason="x WAW lane via carrier"
                )
                # distinct, otherwise-unused columns (4t+s) -- sharing one
                # column creates WAW deps that Tile emits as Pool self-sem
                # waits, overflowing the 1-slot limit.
                for s, sub in enumerate(prev[:-1]):
                    msx = nc.gpsimd.memset(dummy2[:, 4 * t + s : 4 * t + s + 1], 0)
                    add_dep_helper(
                        msx.ins, sub.ins, reason="x WAW lane via carrier"
                    )
                    order(ms3, msx)
                add_dep_helper(
                    ms3.ins,
                    last_mms[t - XBUFS].ins,
                    reason="x slot reuse gated on POOL carrier",
                )
            if t == 0:
                # sub-DMAs: the first k-slices land early so the first
                # matmuls gate on them instead of the full x tile.
                ds = []
                for lo, hi in ((0, 2), (2, 8), (8, 20), (20, 32)):
                    sub = nc.gpsimd.dma_start(
                        x16[:, lo:hi, :], x16_d[0][:, lo:hi, :]
                    )
                    order(sub, ms3)
                    ds.append(sub)
                d8 = nc.gpsimd.dma_start(x8[:], x8_d[0])
                order(d8, ms3)
                ds.append(d8)
                x_dmas.append(ds)
                swdge_all += ds
            else:
                d16 = nc.gpsimd.dma_start(x16[:], x16_d[t])
                order(d16, ms3)
                d8 = nc.gpsimd.dma_start(x8[:], x8_d[t])
                order(d8, ms3)
                x_dmas.append([d16, d8])
                swdge_all += [d16, d8]

            ps = pspool.tile([128, NB, 512], f32)
            # PE carrier: guard LDWEIGHTS absorbing the psum-slot-free (DVE)
            # wait so the first real matmul only waits on PE.
            guard = nc.tensor.ldweights(w16_sb[:, 0, :128])
            if t >= 2:
                add_dep_helper(
                    guard.ins,
                    psum_readers[t - 2].ins,
                    reason="psum slot reuse gated on guard ldweights",
                )

            ob = opool.tile([128, NB, 512], f32)
            # DVE carriers: absorb the ob-slot WAR deps (gen-2 out-store DMA
            # and gen-2 POOL scratch copy) ahead of the epilogue.
            c1 = nc.vector.tensor_copy(dveA[:, t : t + 1], sc_sb[:1, 0, :1])
            c2 = nc.vector.tensor_copy(dveB[:, t : t + 1], sc_sb[:1, 0, :1])
            if t >= 2:
                add_dep_helper(
                    c1.ins, out_dmas[t - 2].ins, reason="ob reuse vs out dma"
                )
                add_dep_helper(
                    c2.ins, out_copies[t - 2].ins, reason="ob reuse vs pool copy"
                )

            if t < TT - 1:
                # Alternate section order by parity (t=1 stays bf16-first to
                # ride the w16 stream): consecutive tiles then share perf
                # mode across the tile boundary.
                bf16_first = (t == 1) or (t % 2 == 0)
                first_mm, last, last_bank = emit_mms(
                    ps, x16, x8, banks=(0, 1, 2, 3), bf16_first=bf16_first
                )
                order(first_mm, guard)
                last_mms.append(last)
                # 1-element DVE carrier reading the last-written psum bank:
                # it absorbs the PE-sem wait so the full-size epilogue ops
                # carry only their own-engine wait.
                pc = nc.vector.tensor_copy(
                    dveC[:, t : t + 1], ps[:1, last_bank, :1]
                )
                mul = nc.vector.tensor_tensor(
                    ob[:], ps[:], sc_sb[:], mybir.AluOpType.mult
                )
                order(mul, pc)
                order(mul, c1)
                order(mul, c2)
                add = nc.vector.tensor_tensor(
                    ob[:], ob[:], bi_sb[:], mybir.AluOpType.add
                )
                order(add, mul)
                psum_readers.append(mul)
                adds.append(add)
                # POOL carrier: RAW on ob -> absorbs the DVE wait ahead of
                # the out-store.
                cp = nc.gpsimd.tensor_copy(scratch[:, t : t + 1], ob[:1, 0, :1])
                od = nc.gpsimd.dma_start(out_d[t], ob[:])
                order(od, cp)
                out_copies.append(cp)
                out_dmas.append(od)
                swdge_all.append(od)
            else:
                # Last tile: bank-by-bank (bf16 banks first), each bank's
                # epilogue+store emitted right after its matmul group, so
                # only one bank's epilogue trails the final matmul.
                prev_ep = None
                for gi, bank in enumerate((3, 2, 1, 0)):
                    first_mm, last, _ = emit_mms(ps, x16, x8, banks=(bank,))
                    if gi == 0:
                        order(first_mm, guard)
                    pcX = nc.vector.tensor_copy(
                        pre[:, 4 + gi : 5 + gi], ps[:1, bank, :1]
                    )
                    if prev_ep is not None:
                        order(pcX, prev_ep)
                    mulX = nc.vector.tensor_tensor(
                        ob[:, bank, :], ps[:, bank, :], sc_sb[:, bank, :],
                        mybir.AluOpType.mult,
                    )
                    order(mulX, pcX)
                    if gi == 0:
                        order(mulX, c1)
                        order(mulX, c2)
                    addX = nc.vector.tensor_tensor(
                        ob[:, bank, :], ob[:, bank, :], bi_sb[:, bank, :],
                        mybir.AluOpType.add,
                    )
                    order(addX, mulX)
                    prev_ep = addX
                    cpX = nc.gpsimd.tensor_copy(
                        dummy3[:, gi : gi + 1], ob[:1, bank, :1]
                    )
                    odX = nc.gpsimd.dma_start(out_d[t][:, bank, :], ob[:, bank, :])
                    order(odX, cpX)
                    swdge_all.append(odX)
                    if gi == 3:
                        last_mms.append(last)
                        psum_readers.append(mulX)
                        adds.append(addX)
                        out_copies.append(cpX)
                        out_dmas.append(odX)

        # Tail carriers: SP nops, one wait each, observing every outstanding
        # sem (PE, DVE, Pool, all SWDGE lanes, preamble HWDGE lanes) so the
        # kernel-tail SP drain doesn't exceed its sync-wait slots.
        tail_deps = [
            last_mms[-1],
            adds[-1],
            out_copies[-1],
        ]
        # HWDGE / SWDGE DMAs stripe over 8 sems each -> covering the last
        # 8 (plus slack) observes every lane's final value.
        tail_deps += hwdge_all[-12:]
        tail_deps += swdge_all[-10:]
        for i, dep in enumerate(tail_deps):
            nop = nc.engines[mybir.EngineType.SP].nop(
                nofuse=True, hint=f"tail_carrier_{i}"
            )
            add_dep_helper(nop.ins, dep.ins, reason="tail drain carrier")

    return nc


def kernel(x, weight_int8, scale, bias):
    global _last_results
    from concourse.bass_utils import run_bass_kernel_spmd

    x = np.asarray(x)
    weight_int8 = np.asarray(weight_int8)
    scale = np.asarray(scale, dtype=np.float32)
    bias = np.asarray(bias, dtype=np.float32)

    # x^T [IN, TOK] in bf16 and e4m3, tiled to [TT, 128p(IN), KT, 128(tok)]
    xT = np.ascontiguousarray(x.reshape(TOK, IN).astype(np.float32).T)
    x_hi = np.ascontiguousarray(
        xT.astype(BF16).reshape(KT, 128, TT, 128).transpose(2, 1, 0, 3)
    )
    x_f8 = np.ascontiguousarray(
        xT.astype(E4M3).reshape(KT, 128, TT, 128).transpose(2, 1, 0, 3)
    )

    # Global row permutation by |scale|: smallest-|scale| rows go to the fp8
    # banks, mid rows to the mixed bank, largest to bf16 (their L2-norm
    # error weight is scale^2).
    order = np.argsort(scale, kind="stable")
    nf8_rows = NCORES * NF8 * 512
    f8_all = order[:nf8_rows]
    mix_all = order[nf8_rows : nf8_rows + NCORES * 512]
    b16_all = order[nf8_rows + NCORES * 512 :]

    def wlayout(c2d, dtype):
        # [rows, IN] -> [128p(k), KT, rows]
        rows = c2d.shape[0]
        return np.ascontiguousarray(
            c2d.astype(dtype).T.reshape(KT, 128, rows).transpose(1, 0, 2)
        )

    in_maps = []
    col_ids = []
    for c in range(NCORES):
        f8_rows = f8_all[c * NF8 * 512 : (c + 1) * NF8 * 512]
        mix_rows = mix_all[c * 512 : (c + 1) * 512]
        b16_rows = b16_all[c * 512 : (c + 1) * 512]
        # psum bank order: 0,1 = pure fp8; 2 = mixed; 3 = pure bf16
        cols = np.concatenate([f8_rows, mix_rows, b16_rows])
        col_ids.append(cols)
        w8 = wlayout(weight_int8[f8_rows].astype(np.float32) * RSC, E4M3)
        w16 = wlayout(weight_int8[b16_rows].astype(np.float32), BF16)
        mixc = weight_int8[mix_rows].astype(np.float32) * RSC
        w8m = wlayout(mixc, E4M3)[:, :MIXK8, :]
        w16m = wlayout(mixc, BF16)[:, MIXK8:, :]
        # 1/RSC folded into the post-matmul scale of the scaled banks (0..2)
        sc_cols = np.concatenate(
            [scale[f8_rows] / RSC, scale[mix_rows] / RSC, scale[b16_rows]]
        )
        sc_bc = np.ascontiguousarray(
            np.broadcast_to(sc_cols, (128, OSH))
        ).reshape(128, NB, 512)
        bi_bc = np.ascontiguousarray(
            np.broadcast_to(bias[cols], (128, OSH))
        ).reshape(128, NB, 512)
        in_maps.append(
            {
                "x16": x_hi,
                "x8": x_f8,
                "w16": w16,
                "w16m": np.ascontiguousarray(w16m),
                "w8": w8,
                "w8m": np.ascontiguousarray(w8m),
                "sc": sc_bc,
                "bi": bi_bc,
            }
        )

    nc = _build_program()
    trace = bool(os.environ.get("KERNEL_TRACE"))
    kwargs = {}
    if trace:
        # Local-only profiling: stub the bucket upload and install the axon
        # NTFF hook (the image's antenv stub lacks axon_hooks).
        import sys
        import types

        from concourse import bass_utils as _bu

        _bu.upload_artifacts = lambda tmpdir: "local://" + tmpdir
        if "antenv.axon_hooks" not in sys.modules:
            import antenv

            mod = types.ModuleType("antenv.axon_hooks")
            _holder = [None]
            mod.set_axon_ntff_profile_hook = lambda h: _holder.__setitem__(0, h)
            mod.get_axon_ntff_profile_hook = lambda: _holder[0]
            antenv.axon_hooks = mod
            sys.modules["antenv.axon_hooks"] = mod
        from antenv.axon_hooks import (
            get_axon_ntff_profile_hook,
            set_axon_ntff_profile_hook,
        )

        if get_axon_ntff_profile_hook() is None:
            from trn_agent_boot.trn_boot import _ntff_profile_via_ctypes

            set_axon_ntff_profile_hook(
                _ntff_profile_via_ctypes(
                    os.environ.get("PJRT_LIBRARY_PATH", "/opt/axon/libaxon_pjrt.so")
                )
            )
        tmpdir = os.environ.get("KERNEL_TRACE_DIR")
        if tmpdir:
            os.makedirs(tmpdir, exist_ok=True)
            kwargs["tmpdir"] = tmpdir

    # One observed run on a thermally-stressed device returned NaNs from a
    # NEFF that is bit-identical to five correct runs -- silent device-level
    # corruption. Retry once on non-finite output.
    for attempt in range(2):
        res = run_bass_kernel_spmd(
            nc,
            in_maps,
            list(range(NCORES)),
            trace=trace,
            **kwargs,
        )
        _last_results = res
        parts = [res.results[c]["out"].reshape(TOK, OSH) for c in range(NCORES)]
        out = np.empty((TOK, OUT), dtype=np.float32)
        out[:, np.concatenate(col_ids)] = np.concatenate(parts, axis=1)
        out = out.reshape(B, S, OUT)
        if np.isfinite(out).all():
            break
    return out
